# revision 1
# baseline (speedup 1.0000x reference)
"""GATNet (4-layer GAT + pooling head) on 8 Trainium2 NeuronCores.

Strategy (per sharding hint): partition nodes across the 8 cores; each core
owns a contiguous block of 2500 (renumbered) nodes and all edges whose dst
lands in that block.  Per layer each core computes its own node table
(h@W plus attention-logit exponentials), the tables are AllGathered into a
full HBM gather table, and each core gathers the source-node rows for its
incoming edges with dma_gather (dst-lane layout: edge slot s of dst-lane p
lands at partition p, free block s).  Edge softmax is restructured so no
per-edge dst-indexed data is ever needed:

    ref:  e = leaky_relu(asrc[src]+adst[dst]); alpha = softmax_dst(e)
    here: m = max(exp(asrc[src]), exp(0.2*asrc[src]) * exp(-0.8*adst[dst]))
          == exp(leaky_relu(asrc+adst)) * exp(-adst)   (dst factor cancels
          in the softmax), with exp(asrc)/exp(0.2*asrc) carried inside the
          gathered row and exp(-0.8*adst) a per-own-node column broadcast.

Segment sums become per-partition free-dim reductions.  All graph index
tables are built on the host (compile-time) from the runtime edge_index.
"""

import os
import sys

import numpy as np

for _p in ("/opt/trn_rl_repo", "/root/.axon_site/_ro/trn_rl_repo"):
    if os.path.isdir(_p) and _p not in sys.path:
        sys.path.insert(0, _p)

import concourse.bacc as bacc
import concourse.bass as bass
import concourse.tile as tile
from concourse import mybir
from concourse.bass_utils import run_bass_kernel_spmd

F32 = mybir.dt.float32
BF16 = mybir.dt.float16  # fp16: 11-bit mantissa, plenty of range here
I16 = mybir.dt.int16
AF = mybir.ActivationFunctionType
ALU = mybir.AluOpType

N_NODES = 20000
N_EDGES = 320000
NCORES = 8
NOWN = N_NODES // NCORES          # 2500
NTILE = (NOWN + 127) // 128       # 20 dst tiles per core
NPAD = NTILE * 128                # 2560 padded own nodes
SLABR = NPAD + 16                 # per-core slab rows (last 16 all-zero)
DUMMY_ROW = NPAD                  # global row 2560 (core 0's first zero row)
TAB_ROWS = NCORES * SLABR         # 20608

# layer configs: (heads, ch, cin, cout)
LCFG = [(8, 8, 16, 64), (8, 8, 64, 64), (8, 8, 64, 64), (4, 64, 64, 256)]
# gather-table row widths (bf16 elems; bytes must be %256==0)
ROWW = [128, 128, 128, 384]       # cols: [hW | A | A2 | pad]
ROWW_H = 64                       # head table: h4 rows

NFUNC = 8000
NFUNC_PAD = 8064                  # 63 chunks of 128
NF_CHUNK = NFUNC_PAD // 128

# edge-phase slot-chunk sizes (free-dim S per gather): keep SBUF bounded
S_CHUNK = [64, 64, 64, 24]


def _idx_layout(flat):
    """Gather idx order i -> SBUF int16 [128, len/16] (pos [i%16, i//16],
    replicated over the 8 q7 core groups)."""
    flat = np.asarray(flat, np.int16)
    assert len(flat) % 16 == 0
    a = flat.reshape(-1, 16).T  # [16, n/16]
    return np.tile(a, (8, 1)).copy()


def _block_diag(a):
    """a [heads, ch] -> [heads*ch, heads] with column h = a[h] on its block."""
    heads, ch = a.shape
    out = np.zeros((heads * ch, heads), np.float32)
    for h in range(heads):
        out[h * ch:(h + 1) * ch, h] = a[h]
    return out


def _preprocess(edge_index, function_idx, flag, decision_var_idxes):
    # NOTE: the appended self-loops are NOT put in the edge tables —
    # the device adds the self term analytically from local node data.
    src = np.asarray(edge_index[0], np.int64)
    dst = np.asarray(edge_index[1], np.int64)

    deg = np.bincount(dst, minlength=N_NODES)

    # renumber: global degree-desc order, round-robin over cores, so every
    # core's tile t covers the same global degree band (minimises slot pad)
    ranks = np.argsort(-deg, kind="stable")
    new_of_orig = np.empty(N_NODES, np.int64)
    gi = np.arange(N_NODES)
    new_of_orig[ranks] = (gi % NCORES) * NOWN + gi // NCORES

    src_n = new_of_orig[src]
    dst_n = new_of_orig[dst]

    # slab row id (p-major within core) for a renumbered-global node id
    def rowid(g):
        c, l = g // NOWN, g % NOWN
        t, p = l // 128, l % 128
        return c * SLABR + p * NTILE + t

    deg_n = np.zeros(N_NODES, np.int64)
    np.add.at(deg_n, dst_n, 1)

    # shared tile slot counts S_t = max over cores of max degree in tile
    degs_2d = deg_n.reshape(NCORES, NOWN)
    S = []
    for t in range(NTILE):
        hi = min((t + 1) * 128, NOWN)
        S.append(int(degs_2d[:, t * 128:hi].max()))
    S = [max(s, 1) for s in S]

    # per-core edge slot tables A[l, s] = rowid(src) (DUMMY_ROW pad)
    order = np.argsort(dst_n, kind="stable")
    dst_s, src_s = dst_n[order], src_n[order]
    starts = np.searchsorted(dst_s, np.arange(N_NODES))
    slot = np.arange(len(dst_s)) - starts[dst_s]
    src_row = rowid(src_s)

    idx_tabs = []
    for c in range(NCORES):
        m = (dst_s >= c * NOWN) & (dst_s < (c + 1) * NOWN)
        l = dst_s[m] - c * NOWN
        A = np.full((NPAD, max(S)), DUMMY_ROW, np.int64)
        A[l, slot[m]] = src_row[m]
        parts = []
        for t in range(NTILE):
            blk = A[t * 128:(t + 1) * 128, :S[t]].T  # [S_t, 128]
            parts.append(blk.reshape(-1))
        idx_tabs.append(_idx_layout(np.concatenate(parts)))

    # head tables: per-core partial pools over OWN nodes (local slab rows)
    def local_row(g):
        l = g % NOWN
        return (l % 128) * NTILE + l // 128

    fidx_new = new_of_orig[np.asarray(function_idx, np.int64)]
    fowner = fidx_new // NOWN
    flg = np.asarray(flag, np.int64)
    counts = np.bincount(fowner, minlength=NCORES)
    nfp = ((int(counts.max()) + 127) // 128) * 128
    idx_funcs, fonehots = [], []
    for c in range(NCORES):
        m = fowner == c
        rows = np.full(nfp, DUMMY_ROW, np.int64)
        rows[:counts[c]] = local_row(fidx_new[m])
        idx_funcs.append(_idx_layout(rows))
        fo = np.zeros((nfp, 8), np.float32)
        fo[np.arange(counts[c]), flg[m]] = 1.0
        fonehots.append(fo.reshape(nfp // 128, 128, 8)
                        .transpose(1, 0, 2).copy())

    dvn = new_of_orig[np.asarray(decision_var_idxes, np.int64)]
    downer = dvn // NOWN
    idx_dvs, dvones = [], []
    for c in range(NCORES):
        rows = np.full(128, DUMMY_ROW, np.int64)
        dvo = np.zeros((128, 8), np.float32)
        k = 0
        for j in range(NCORES):
            if downer[j] == c:
                rows[k] = local_row(dvn[j])
                dvo[k, j] = 1.0
                k += 1
        idx_dvs.append(_idx_layout(rows))
        dvones.append(dvo)

    return dict(
        new_of_orig=new_of_orig, S=S, idx_tabs=idx_tabs, nf_chunk=nfp // 128,
        idx_funcs=idx_funcs, fonehots=fonehots, idx_dvs=idx_dvs,
        dvones=dvones,
    )


def _build_kernel(S, idx_width, nfc):
    """Build the SPMD bass program (same for all cores).

    Per-layer gather-table rows are uniformly 128 fp16 (=256B):
    [feat(64) | asrc(h) | adst(h) | 0-pad], where feat is hW for layers
    1-3 and raw h3 for layer 4 (W4 is applied after aggregation via
    linearity: sum_e m_e*(h3@W4) == (sum_e m_e*h3)@W4).
    """
    nc = bacc.Bacc("TRN2", target_bir_lowering=False, debug=False,
                   num_devices=NCORES)

    # ---- external inputs ----
    xT = nc.dram_tensor("xT", [16, NPAD], F32, kind="ExternalInput")
    idx_edges = nc.dram_tensor("idx_edges", [128, idx_width], I16,
                               kind="ExternalInput")
    wext = [nc.dram_tensor(f"wext{l}", [64 if l else 16, 80], F32,
                           kind="ExternalInput") for l in range(3)]
    wsd4_d = nc.dram_tensor("wsd4", [64, 8], F32, kind="ExternalInput")
    wst_d = nc.dram_tensor("wst", [128, 2, 64], F32, kind="ExternalInput")
    biases = [nc.dram_tensor(f"bias{l}", [128, 64], F32, kind="ExternalInput")
              for l in range(4)]
    fonehot_d = nc.dram_tensor("fonehot", [128, nfc, 8], F32,
                               kind="ExternalInput")
    idx_func_d = nc.dram_tensor("idx_func", [128, nfc * 8], I16,
                                kind="ExternalInput")
    idx_dv_d = nc.dram_tensor("idx_dv", [128, 8], I16, kind="ExternalInput")
    dvone_d = nc.dram_tensor("dvone", [128, 8], F32, kind="ExternalInput")
    wp_d = nc.dram_tensor("wp", [64, 64], F32, kind="ExternalInput")
    wt_d = nc.dram_tensor("wt", [64, 64], F32, kind="ExternalInput")
    wo_d = nc.dram_tensor("wo", [128, 1], F32, kind="ExternalInput")
    bo_d = nc.dram_tensor("bo", [128, 1], F32, kind="ExternalInput")
    ident_d = nc.dram_tensor("ident", [128, 128], F32, kind="ExternalInput")
    padmask_d = nc.dram_tensor("padmask", [128, 1], F32, kind="ExternalInput")

    out_final = nc.dram_tensor("out_final", [8, 1], F32, kind="ExternalOutput")
    z4_out = nc.dram_tensor("z4_out", [128, NTILE * 64], F32,
                            kind="ExternalOutput")

    with tile.TileContext(nc) as tc:
        with (
            tc.tile_pool(name="dram", bufs=1, space="DRAM") as dram,
            tc.tile_pool(name="const", bufs=1) as cpool,
            tc.tile_pool(name="state", bufs=1) as spool,
            tc.tile_pool(name="gather", bufs=3) as gpool,
            tc.tile_pool(name="msg", bufs=3) as mpool,
            tc.tile_pool(name="small", bufs=6) as tpool,
            tc.tile_pool(name="psum", bufs=2, space="PSUM") as ppool,
            tc.tile_pool(name="psacc", bufs=1, space="PSUM") as papool,
            tc.tile_pool(name="psumT", bufs=2, space="PSUM") as ptpool,
        ):
            # ---- DRAM internals ----
            slab123 = dram.tile([SLABR, 128], BF16, tag="slab123")
            slab4 = dram.tile([SLABR, 128], BF16, tag="slab4")
            slabH = dram.tile([SLABR, 64], F32, tag="slabH")
            tabs = []
            for _l in range(4):
                tab_l = dram.tile([TAB_ROWS, 128], BF16, tag=f"tab{_l}",
                                  addr_space="Shared", name=f"tab{_l}")
                tabs.append(tab_l)

            # ---- load constants ----
            w_sb = []
            for l in range(3):
                t = cpool.tile([64 if l else 16, 80], F32, tag=f"w{l}")
                nc.sync.dma_start(t[:], wext[l][:, :])
                w_sb.append(t)
            wsd4_sb = cpool.tile([64, 8], F32, tag="wsd4")
            nc.sync.dma_start(wsd4_sb[:], wsd4_d[:, :])
            wst_sb = cpool.tile([128, 2, 64], F32, tag="wst")
            nc.sync.dma_start(wst_sb[:], wst_d[:, :, :])
            b_sb = []
            for l in range(4):
                t = cpool.tile([128, 64], F32, tag=f"b{l}")
                nc.sync.dma_start(t[:], biases[l][:, :])
                b_sb.append(t)
            idxe_sb = cpool.tile([128, idx_width], I16, tag="idxe")
            nc.sync.dma_start(idxe_sb[:], idx_edges[:, :])
            fone_sb = cpool.tile([128, nfc, 8], F32, tag="fone")
            nc.sync.dma_start(fone_sb[:], fonehot_d[:, :, :])
            idxf_sb = cpool.tile([128, nfc * 8], I16, tag="idxf")
            nc.sync.dma_start(idxf_sb[:], idx_func_d[:, :])
            idxdv_sb = cpool.tile([128, 8], I16, tag="idxdv")
            nc.sync.dma_start(idxdv_sb[:], idx_dv_d[:, :])
            dvone_sb = cpool.tile([128, 8], F32, tag="dvone")
            nc.sync.dma_start(dvone_sb[:], dvone_d[:, :])
            wp_sb = cpool.tile([64, 64], F32, tag="wp")
            nc.sync.dma_start(wp_sb[:], wp_d[:, :])
            wt_sb = cpool.tile([64, 64], F32, tag="wt")
            nc.sync.dma_start(wt_sb[:], wt_d[:, :])
            wo_sb = cpool.tile([128, 1], F32, tag="wo")
            nc.sync.dma_start(wo_sb[:], wo_d[:, :])
            bo_sb = cpool.tile([128, 1], F32, tag="bo")
            nc.sync.dma_start(bo_sb[:], bo_d[:, :])
            ident_sb = cpool.tile([128, 128], F32, tag="ident")
            nc.sync.dma_start(ident_sb[:], ident_d[:, :])
            padmask_sb = cpool.tile([128, 1], F32, tag="padmask")
            nc.sync.dma_start(padmask_sb[:], padmask_d[:, :])

            zero_sb = cpool.tile([128, 128], BF16, tag="zero")
            nc.vector.memset(zero_sb[:], 0.0)
            negone = cpool.tile([128, 1], F32, tag="negone")
            nc.vector.memset(negone[:], -1.0)
            # zero the trailing pad rows of each slab (gather dummy target)
            nc.sync.dma_start(slab123[NPAD:SLABR, :], zero_sb[0:16, :])
            nc.sync.dma_start(slab4[NPAD:SLABR, :], zero_sb[0:16, :])
            nc.sync.dma_start(slabH[NPAD:SLABR, :],
                              zero_sb[0:16, :].bitcast(F32))

            # ---- persistent state ----
            zT = spool.tile([64, NPAD], F32, tag="zT")
            nc.vector.memset(zT[:], 0.0)
            nc.sync.dma_start(zT[0:16, :], xT[:, :])

            ntab_sb = spool.tile([128, NTILE, 128], BF16, tag="ntab")
            nc.vector.memset(ntab_sb[:], 0.0)
            msel = spool.tile([128, NTILE, 8], F32, tag="msel")
            zbuf = spool.tile([128, NTILE, 256], F32, tag="zbuf")
            zm = spool.tile([128, NTILE, 64], F32, tag="zm")
            cvals = spool.tile([128, NTILE, 8], BF16, tag="cvals")
            denom = spool.tile([128, NTILE, 8], F32, tag="denom")
            drec = spool.tile([128, NTILE, 8], F32, tag="drec")

            col0 = np.cumsum([0] + [8 * s for s in S]).tolist()

            def node_tile(l, t):
                """ntab rows [feat|asrc|adst] + exps + slab write, tile t."""
                h = LCFG[l][0]
                if l < 3:
                    cin = 64 if l else 16
                    ps = ppool.tile([128, 80], F32, tag="pnode")
                    nc.tensor.matmul(ps[:], zT[0:cin, t * 128:(t + 1) * 128],
                                     w_sb[l][:], start=True, stop=True)
                    nc.scalar.copy(ntab_sb[:, t, 0:80], ps[:])
                else:
                    nc.scalar.copy(ntab_sb[:, t, 0:64], zm[:, t, :])
                    ps = ppool.tile([128, 8], F32, tag="pnode")
                    nc.tensor.matmul(ps[:], zT[0:64, t * 128:(t + 1) * 128],
                                     wsd4_sb[:], start=True, stop=True)
                    nc.scalar.copy(ntab_sb[:, t, 64:72], ps[:])
                asrc = ntab_sb[:, t, 64:64 + h]
                adst = ntab_sb[:, t, 64 + h:64 + 2 * h]
                nc.scalar.activation(cvals[:, t, 0:h], adst, AF.Exp,
                                     scale=-0.8)
                nc.scalar.activation(adst, asrc, AF.Exp, scale=0.2)
                nc.scalar.activation(asrc, asrc, AF.Exp)
                slab = slab123 if l < 3 else slab4
                slab_ap = slab[0:NPAD, :].rearrange(
                    "(p t) f -> p t f", t=NTILE)[:, t, :]
                nc.sync.dma_start(slab_ap, ntab_sb[:, t, :])

            def finish_tables(l):
                slab = slab123 if l < 3 else slab4
                nc.gpsimd.collective_compute(
                    "AllGather", ALU.bypass,
                    replica_groups=[list(range(NCORES))],
                    ins=[slab[:, :].opt()],
                    outs=[tabs[l][:, :].opt()],
                )

            for t in range(NTILE):
                node_tile(0, t)
            finish_tables(0)

            for l in range(4):
                heads, ch, cin, cout = LCFG[l]
                mw = heads * ch              # zbuf width (64 / 256)
                s2 = S_CHUNK[l]
                tab = tabs[l]

                for t in range(NTILE):
                    # ---- self-loop term initialises zbuf/denom ----
                    nc.vector.tensor_tensor(
                        msel[:, t, 0:heads],
                        ntab_sb[:, t, 64 + heads:64 + 2 * heads],
                        cvals[:, t, 0:heads], ALU.mult)
                    nc.vector.tensor_tensor(
                        msel[:, t, 0:heads],
                        ntab_sb[:, t, 64:64 + heads],
                        msel[:, t, 0:heads], ALU.max)
                    nc.scalar.copy(denom[:, t, 0:heads],
                                   msel[:, t, 0:heads])
                    if l < 3:
                        sf = ntab_sb[:, t, 0:64].rearrange(
                            "p (h c) -> p h c", h=heads)
                    else:
                        sf = ntab_sb[:, t, 0:64].unsqueeze(1).broadcast_to(
                            [128, heads, ch])
                    nc.vector.tensor_tensor(
                        zbuf[:, t, 0:mw].rearrange("p (h c) -> p h c",
                                                   h=heads),
                        sf,
                        msel[:, t, 0:heads].unsqueeze(2).broadcast_to(
                            [128, heads, ch]), ALU.mult)

                    # ---- edge chunks ----
                    for s0 in range(0, S[t], s2):
                        sn = min(s2, S[t] - s0)
                        g = gpool.tile([128, s2, 128], BF16, tag="g")
                        icols = idxe_sb[:, col0[t] + 8 * s0:
                                        col0[t] + 8 * (s0 + sn)]
                        nc.gpsimd.dma_gather(
                            g[:, 0:sn, :], tab[:, :], icols,
                            sn * 128, sn * 128, 128, single_packet=False)
                        mt = tpool.tile([128, s2, 8], BF16, tag="mt")
                        cb = cvals[:, t, 0:heads].unsqueeze(1)
                        cb = cb.broadcast_to([128, sn, heads])
                        nc.vector.tensor_tensor(
                            mt[:, 0:sn, 0:heads],
                            g[:, 0:sn, 64 + heads:64 + 2 * heads],
                            cb, ALU.mult)
                        nc.vector.tensor_tensor(
                            mt[:, 0:sn, 0:heads],
                            g[:, 0:sn, 64:64 + heads],
                            mt[:, 0:sn, 0:heads], ALU.max)
                        dt = tpool.tile([128, 8], F32, tag="dt")
                        nc.vector.tensor_reduce(
                            dt[:, 0:heads],
                            mt[:, 0:sn, 0:heads].transpose([0, 2, 1]),
                            mybir.AxisListType.X, ALU.add)
                        nc.vector.tensor_tensor(
                            denom[:, t, 0:heads], denom[:, t, 0:heads],
                            dt[:, 0:heads], ALU.add)
                        mg = mpool.tile([128, s2, mw], BF16, tag="mg")
                        if l < 3:
                            gf = g[:, 0:sn, 0:64].rearrange(
                                "p s (h c) -> p s h c", h=heads)
                        else:
                            gf = g[:, 0:sn, 0:64].unsqueeze(2).broadcast_to(
                                [128, sn, heads, ch])
                        nc.vector.tensor_tensor(
                            mg[:, 0:sn, :].rearrange(
                                "p s (h c) -> p s h c", h=heads),
                            gf,
                            mt[:, 0:sn, 0:heads].unsqueeze(3).broadcast_to(
                                [128, sn, heads, ch]), ALU.mult)
                        ot = tpool.tile([128, 256], F32, tag="ot")
                        nc.vector.tensor_reduce(
                            ot[:, 0:mw],
                            mg[:, 0:sn, :].transpose([0, 2, 1]),
                            mybir.AxisListType.X, ALU.add)
                        nc.vector.tensor_tensor(
                            zbuf[:, t, 0:mw], zbuf[:, t, 0:mw],
                            ot[:, 0:mw], ALU.add)

                    # ---- per-tile epilogue ----
                    # (denom >= A_self = exp(asrc) > 0: no clamp needed)
                    nc.vector.reciprocal(drec[:, t, 0:heads],
                                         denom[:, t, 0:heads])
                    zt4 = zbuf[:, t, 0:mw].rearrange("p (h c) -> p h c",
                                                     h=heads)
                    nc.vector.tensor_tensor(
                        zt4, zt4,
                        drec[:, t, 0:heads].unsqueeze(2).broadcast_to(
                            [128, heads, ch]), ALU.mult)
                    if l < 3:
                        nc.scalar.copy(zm[:, t, :], zbuf[:, t, 0:64])
                    else:
                        # z4pre = u_cat @ Wst (K=256 via 2 chunks)
                        pz = papool.tile([128, 64], F32, tag="pz4")
                        for j in range(2):
                            ut = ptpool.tile([128, 128], F32, tag="pt")
                            nc.tensor.transpose(
                                ut[:], zbuf[:, t, j * 128:(j + 1) * 128],
                                ident_sb[:])
                            us = tpool.tile([128, 128], F32, tag="us")
                            nc.scalar.copy(us[:], ut[:])
                            nc.tensor.matmul(pz[:], us[:], wst_sb[:, j, :],
                                             start=(j == 0), stop=(j == 1))
                        nc.scalar.copy(zm[:, t, :], pz[:])
                    # bias + elu(x) = relu(x) + exp(-relu(-x)) - 1
                    e1 = tpool.tile([128, 64], F32, tag="e1")
                    e2 = tpool.tile([128, 64], F32, tag="e2")
                    nc.vector.tensor_tensor(zm[:, t, :], zm[:, t, :],
                                            b_sb[l][:, :], ALU.add)
                    nc.scalar.activation(e1[:], zm[:, t, :], AF.Relu)
                    nc.scalar.activation(e2[:], zm[:, t, :], AF.Relu,
                                         scale=-1.0)
                    nc.scalar.activation(e2[:], e2[:], AF.Exp, scale=-1.0)
                    nc.vector.tensor_tensor(zm[:, t, :], e1[:], e2[:],
                                            ALU.add)
                    nc.scalar.activation(zm[:, t, :], zm[:, t, :], AF.Identity,
                                         bias=negone[:, 0:1])
                    if t == NTILE - 1 and NOWN % 128:
                        nc.scalar.mul(zm[:, t, :], zm[:, t, :],
                                      padmask_sb[:, 0:1])
                    if l < 3:
                        pt = ptpool.tile([64, 128], F32, tag="pt")
                        nc.tensor.transpose(pt[:], zm[:, t, :], ident_sb[:])
                        nc.scalar.copy(zT[0:64, t * 128:(t + 1) * 128],
                                       pt[:])
                        node_tile(l + 1, t)

                if l < 3:
                    finish_tables(l + 1)

            # ---- final h4 -> local head table + debug out ----
            slabH_ap = slabH[0:NPAD, :].rearrange("(p t) f -> p t f",
                                                  t=NTILE)
            nc.sync.dma_start(slabH_ap, zm[:, :, :])
            nc.sync.dma_start(z4_out[:, :],
                              zm[:, :, :].rearrange("p t f -> p (t f)"))

            # ---- head: per-core partial pools, then one small AllReduce ----
            fg = gpool.tile([128, nfc, 64], F32, tag="fg")
            nc.gpsimd.dma_gather(fg[:, 0:nfc, :], slabH[:, :],
                                 idxf_sb[:], nfc * 128, nfc * 128, 64,
                                 single_packet=False)
            fps = papool.tile([8, 64], F32, tag="fps")
            for k in range(nfc):
                nc.tensor.matmul(fps[:], fone_sb[:, k, :], fg[:, k, :],
                                 start=(k == 0), stop=(k == nfc - 1))
            dg = tpool.tile([128, 1, 64], F32, tag="dg")
            nc.gpsimd.dma_gather(dg[:], slabH[:, :], idxdv_sb[:],
                                 128, 128, 64)
            tgp = ppool.tile([8, 64], F32, tag="h2nd")
            nc.tensor.matmul(tgp[:], dvone_sb[:], dg[:, 0, :],
                             start=True, stop=True)
            hbuf = tpool.tile([8, 128], F32, tag="hbuf")
            nc.scalar.copy(hbuf[:, 0:64], fps[:])
            nc.scalar.copy(hbuf[:, 64:128], tgp[:])
            arin = dram.tile([8, 128], F32, tag="arin")
            arout = dram.tile([8, 128], F32, tag="arout",
                              addr_space="Shared")
            nc.sync.dma_start(arin[:, :], hbuf[:])
            nc.gpsimd.collective_compute(
                "AllReduce", ALU.add,
                replica_groups=[list(range(NCORES))],
                ins=[arin[:, :].opt()],
                outs=[arout[:, :].opt()],
            )
            hb2 = tpool.tile([8, 128], F32, tag="hb2")
            nc.sync.dma_start(hb2[:], arout[:, :])

            def transpose_8x64(src_ap, tag):
                s8 = tpool.tile([8, 64], F32, tag=tag + "s")
                nc.scalar.copy(s8[:], src_ap)
                pt = ptpool.tile([64, 8], F32, tag="pt")
                nc.tensor.transpose(pt[:], s8[:], ident_sb[0:8, 0:8])
                o = tpool.tile([64, 8], F32, tag=tag + "o")
                nc.scalar.copy(o[:], pt[:])
                return o

            fpT = transpose_8x64(hb2[:, 0:64], "fp")
            tgT = transpose_8x64(hb2[:, 64:128], "tg")

            fp2 = ppool.tile([8, 64], F32, tag="h2nd")
            nc.tensor.matmul(fp2[:], fpT[:], wp_sb[:], start=True, stop=True)
            tg2 = ppool.tile([8, 64], F32, tag="h2nd")
            nc.tensor.matmul(tg2[:], tgT[:], wt_sb[:], start=True, stop=True)

            zh = tpool.tile([128, 8], F32, tag="zh")
            f2T = transpose_8x64(fp2[:], "f2")
            t2T = transpose_8x64(tg2[:], "t2")
            nc.scalar.copy(zh[0:64, :], f2T[:])
            nc.scalar.copy(zh[64:128, :], t2T[:])
            h1 = tpool.tile([128, 8], F32, tag="h1")
            h2 = tpool.tile([128, 8], F32, tag="h2")
            nc.scalar.activation(h1[:], zh[:], AF.Relu)
            nc.scalar.activation(h2[:], zh[:], AF.Relu, scale=-1.0)
            nc.scalar.activation(h2[:], h2[:], AF.Exp, scale=-1.0)
            nc.vector.tensor_tensor(zh[:], h1[:], h2[:], ALU.add)
            nc.scalar.activation(zh[:], zh[:], AF.Identity,
                                 bias=negone[:, 0:1])

            fin = ppool.tile([8, 1], F32, tag="h2nd")
            nc.tensor.matmul(fin[:], zh[:], wo_sb[:], start=True, stop=True)
            fo = tpool.tile([8, 1], F32, tag="fo")
            nc.scalar.activation(fo[:], fin[:], AF.Identity, bias=bo_sb[0:8, :])
            nc.sync.dma_start(out_final[:, :], fo[:])

    nc.compile()
    return nc


def prepare(x, edge_index, function_idx, flag, decision_var_idxes,
            W1, a_src1, a_dst1, b1, W2, a_src2, a_dst2, b2,
            W3, a_src3, a_dst3, b3, W4, a_src4, a_dst4, b4,
            Wp, Wt, Wo, bo):
    """Host preprocessing + program build -> (nc, in_maps)."""
    x = np.asarray(x, np.float32)
    pp = _preprocess(edge_index, function_idx, flag, decision_var_idxes)
    S, idx_tabs = pp["S"], pp["idx_tabs"]
    idx_width = idx_tabs[0].shape[1]

    nc = _build_kernel(S, idx_width, pp["nf_chunk"])

    # host-side input prep
    wa = [(W1, a_src1, a_dst1), (W2, a_src2, a_dst2),
          (W3, a_src3, a_dst3)]
    wexts = []
    for l, (W, asr, ads) in enumerate(wa):
        W = np.asarray(W, np.float32)
        asr = np.asarray(asr, np.float32)
        ads = np.asarray(ads, np.float32)
        wexts.append(np.concatenate(
            [W, W @ _block_diag(asr), W @ _block_diag(ads)], axis=1
        ).astype(np.float32))
    W4 = np.asarray(W4, np.float32)
    wsd4 = np.concatenate(
        [W4 @ _block_diag(np.asarray(a_src4, np.float32)),
         W4 @ _block_diag(np.asarray(a_dst4, np.float32))],
        axis=1).astype(np.float32)                     # [64, 8]
    wst = (W4.reshape(64, 4, 64).transpose(1, 0, 2).reshape(256, 64)
           / 4.0)                                      # [256, 64], mean folded
    wst = wst.reshape(2, 128, 64).transpose(1, 0, 2).copy()  # [128, 2, 64]
    bs = [np.tile(np.asarray(b, np.float32)[None, :], (128, 1))
          for b in (b1, b2, b3, b4)]

    new_of_orig = pp["new_of_orig"]
    orig_of_new = np.empty(N_NODES, np.int64)
    orig_of_new[new_of_orig] = np.arange(N_NODES)

    shared = {
        "wext0": wexts[0], "wext1": wexts[1], "wext2": wexts[2],
        "wsd4": wsd4, "wst": wst,
        "bias0": bs[0], "bias1": bs[1], "bias2": bs[2], "bias3": bs[3],
        "wp": np.asarray(Wp, np.float32), "wt": np.asarray(Wt, np.float32),
        "wo": np.asarray(Wo, np.float32).reshape(128, 1),
        "bo": np.full((128, 1), np.float32(np.asarray(bo).reshape(-1)[0])),
        "ident": np.eye(128, dtype=np.float32),
        "padmask": (np.arange(128) < (NOWN % 128 or 128)
                    ).astype(np.float32).reshape(128, 1),
    }
    in_maps = []
    for c in range(NCORES):
        xT = np.zeros((16, NPAD), np.float32)
        xo = x[orig_of_new[c * NOWN:(c + 1) * NOWN]]  # [2500,16] local order
        xT[:, 0:NOWN] = xo.T
        m = dict(shared)
        m["xT"] = xT
        m["idx_edges"] = idx_tabs[c]
        m["fonehot"] = pp["fonehots"][c]
        m["idx_func"] = pp["idx_funcs"][c]
        m["idx_dv"] = pp["idx_dvs"][c]
        m["dvone"] = pp["dvones"][c]
        in_maps.append(m)
    return nc, in_maps


def kernel(**inputs):
    nc, in_maps = prepare(**inputs)
    trace = os.environ.get("GAT_TRACE", "0") == "1"
    res = run_bass_kernel_spmd(nc, in_maps, core_ids=list(range(NCORES)),
                               trace=trace)
    global last_results
    last_results = res
    out = res.results[0]["out_final"].astype(np.float32)
    return out


last_results = None

